# revision 39
# baseline (speedup 1.0000x reference)
"""EntropyGraph Trainium2 kernel (v2).

Computes, per batch b (one NeuronCore per batch):
  qt = heads(queries @ Wq_w.T + Wq_b), kt = heads(keys @ Wk_w.T + Wk_b)
  out[b,h,i,j] = -0.5 * sum_m log(1 - corr_m(i,j)^2 + eps)
where corr_m is the lag-m cross-correlation between query series i and key
series j within each head.

Structure vs v1:
  - corr = alpha_i * G[i,j]; G = PE Gram of (raw q rows + mean-aug row)
    against (beta-scaled k rows + -s1y-aug rows). One-sided centering makes
    the mean correction exact.
  - Per iteration t = 8h+ic the two Gram PSUM tiles are evacuated as
      v1 = (a1*G1)^2  (ACT Square, scale=a1 per partition)
      v2: mostly DVE tensor_scalar (r2 = a2*G2, PSUM allows one operand)
          followed by an f16 square on DVE/Pool; ~6/16 slots ride ACT
          Square instead to balance engine load
    then u = (v1-C)(v2-C) as quad-batched [128,4096] f16 DVE ops
    (tensor_scalar 4x + tensor_tensor 2x; scalar_tensor_tensor has no DVE
    perf modes so it is avoided), one ACT Ln per quad, and a -0.5 DVE
    tensor_scalar (4x) written back over the dead u tile.
  - beta broadcast comes from a PE outer product (mask @ beta16) into
    PSUM instead of SWDGE DRAM broadcasts: -4 MB DMA traffic and no
    betad bounce on the startup critical path.
  - All ACT functions (Identity/Copy/Square/Ln/Exp) live in one table set
    (natural_log_exp_and_others, forced via _ActTablePatch); rsqrt is
    computed as exp(-0.5*ln(x)) so no table switches occur.
  - Inputs are cast to f32r so every matmul runs at 1 cycle/row.
  - Prologue runs per side (k fully first: its stats feed beta -> bb ->
    yt which gate head 0) with squared-projection tiles split ACT/DVE.
"""

import sys

import numpy as np

sys.path.insert(0, "/opt/trn_rl_repo")

import concourse.bacc as bacc
import concourse.tile as tile
from concourse import mybir
from concourse.bass_utils import run_bass_kernel_spmd

F32 = mybir.dt.float32
F32R = mybir.dt.float32r
F16 = mybir.dt.float16
OP = mybir.AluOpType
AF = mybir.ActivationFunctionType

B, N, DF = 8, 1024, 128
H, DK = 8, 64
EPS = 1e-6
C = 1.0 + EPS
NCHUNK = 4
# evac2 rides ACT (instead of DVE) when t % 16 in this set: engine balance
_ACT_EVAC2_SLOTS = (1, 3, 6, 9, 12, 14)
# on the DVE evac2 path, the f16 squaring op goes to Pool when t % 16 in
# this set (Pool Multiply eff 0.42 but otherwise idle in steady state)
_POOL_SQ_SLOTS = (0, 2, 4, 5, 7, 8, 10, 11, 13, 15)
# -0.5 scale quads routed to Pool when qd % 4 in this set
_POOL_SCALE_SLOTS = ()


def _emit_body(nc, tc, t):
    qT, kT, wqT, wkT, bq, bk, xmask, ymask, invn, ident, out, bbm = t
    with tc.tile_pool(name="const", bufs=1) as const, \
         tc.tile_pool(name="proj", bufs=1) as projp, \
         tc.tile_pool(name="stats", bufs=1) as statp:

        # statp: tiles that stage E reads; everything else transient.
        ns1y = statp.tile([16, N], F32R)
        mx = statp.tile([16, N], F32R)
        aT = statp.tile([128, 128], F32)
        beta16 = statp.tile([16, N], F32R)

        invn_s = const.tile([16, 1], F32)
        id_s = const.tile([128, 128], F32)
        bbm_r = const.tile([16, 8 * 128], F32R)

        with tc.tile_pool(name="inp", bufs=1) as inp, \
             tc.tile_pool(name="statd", bufs=1) as statd:
            # ---- Stage A: load inputs ---------------------------------
            qT_s = inp.tile([DF, N], F32)
            kT_s = inp.tile([DF, N], F32)
            wqT_s = inp.tile([DF, 512], F32)
            wkT_s = inp.tile([DF, 512], F32)
            bq_s = inp.tile([128, 4], F32)
            bk_s = inp.tile([128, 4], F32)
            xm_s = inp.tile([128, 64], F32)
            ym_s = inp.tile([128, 64], F32)
            bbm_s = inp.tile([16, 8 * 128], F32)
            for dst, src in ((kT_s, kT), (wkT_s, wkT), (qT_s, qT),
                             (wqT_s, wqT), (bq_s, bq), (bk_s, bk),
                             (xm_s, xmask), (ym_s, ymask), (invn_s, invn),
                             (id_s, ident), (bbm_s, bbm)):
                nc.sync.dma_start(out=dst, in_=src[:, :])

            # f32r rounding casts (the verifier rejects raw-DMA data as
            # f32r matmul input). k-side first: beta gates stage E head 0.
            kT_r = inp.tile([DF, N], F32R)
            wkT_r = inp.tile([DF, 512], F32R)
            qT_r = inp.tile([DF, N], F32R)
            wqT_r = inp.tile([DF, 512], F32R)
            xm_r = inp.tile([128, 64], F32R)
            ym_r = inp.tile([128, 64], F32R)
            nc.vector.tensor_copy(kT_r, kT_s)
            nc.scalar.copy(wkT_r, wkT_s)
            nc.vector.tensor_copy(qT_r, qT_s)
            nc.scalar.copy(wqT_r, wqT_s)
            nc.scalar.copy(ym_r, ym_s)
            nc.scalar.copy(xm_r, xm_s)
            nc.scalar.copy(bbm_r, bbm_s)

            # ---- Stages B-D, one side at a time ----------------------
            # Each side runs proj -> sq -> moment matmuls -> stats evac ->
            # stage-D math end-to-end, k-side first: the k chain feeds
            # beta16 -> bb -> yt (head 0 Gram rhs) while the q side is
            # still projecting, and every engine queue sees the k-chain
            # ops first. sq ops stay off Pool so bb broadcasts are not
            # stuck behind 2.1us Pool multiplies.
            qproj = []
            kproj = []
            stats_sb = {}
            with tc.tile_pool(name="sqp", bufs=1) as sqp, \
                 tc.tile_pool(name="sps", bufs=1, space="PSUM") as sps:
                for (src_r, w_r, b_s, mask, dst_list, pname) in (
                        (kT_r, wkT_r, bk_s, ym_r, kproj, "k"),
                        (qT_r, wqT_r, bq_s, xm_r, qproj, "q")):
                    sq_side = []
                    with tc.tile_pool(name=f"pps{pname}", bufs=2,
                                      space="PSUM") as pps:
                        for c in range(NCHUNK):
                            psb = pps.tile([128, N], F32, tag="pps")
                            for jh in range(2):
                                nc.tensor.matmul(
                                    psb[:, jh * 512:(jh + 1) * 512],
                                    lhsT=w_r[:, c * 128:(c + 1) * 128],
                                    rhs=src_r[:, jh * 512:(jh + 1) * 512],
                                    start=True, stop=True)
                            pt = projp.tile([128, N], F32R,
                                            tag=f"proj_{pname}_{c}")
                            if c % 2 == 1:
                                nc.vector.tensor_scalar(
                                    out=pt, in0=psb, scalar1=1.0,
                                    scalar2=b_s[:, c:c + 1],
                                    op0=OP.mult, op1=OP.add)
                            else:
                                nc.scalar.activation(
                                    out=pt, in_=psb, func=AF.Identity,
                                    bias=b_s[:, c:c + 1], scale=1.0)
                            dst_list.append(pt)
                            sq = sqp.tile([128, N], F32R,
                                          tag=f"sq{pname}{c}")
                            if c % 2 == 0:
                                nc.vector.tensor_mul(sq, pt, pt)
                            else:
                                nc.scalar.activation(
                                    out=sq, in_=pt, func=AF.Square,
                                    bias=0.0, scale=1.0)
                            sq_side.append(sq)
                            # moment matmuls interleave per chunk so the
                            # stats finish one evac+sq after the last chunk
                            if c == 0:
                                ps1 = sps.tile([16, N], F32, tag="ps1")
                                ps2 = sps.tile([16, N], F32, tag="ps2")
                            for jh in range(2):
                                sl = slice(jh * 512, (jh + 1) * 512)
                                nc.tensor.matmul(
                                    ps1[:, sl],
                                    lhsT=mask[:, 16 * c:16 * c + 16],
                                    rhs=pt[:, sl],
                                    start=(c == 0), stop=(c == NCHUNK - 1))
                                nc.tensor.matmul(
                                    ps2[:, sl],
                                    lhsT=mask[:, 16 * c:16 * c + 16],
                                    rhs=sq[:, sl],
                                    start=(c == 0), stop=(c == NCHUNK - 1))

                    s1 = statd.tile([16, N], F32, tag=f"s1{pname}")
                    s2 = statd.tile([16, N], F32, tag=f"s2{pname}")
                    nc.scalar.copy(s1, ps1)
                    nc.vector.tensor_copy(s2, ps2)
                    stats_sb[pname] = (s1, s2)

                    invn_ap = invn_s[:, 0:1]
                    if pname == "k":
                        # nssy = s1y^2/n - s2y = -ssy; beta = exp(-.5 ln ssy)
                        nc.vector.tensor_scalar(out=ns1y, in0=s1,
                                                scalar1=-1.0, scalar2=None,
                                                op0=OP.mult)
                        tk = statd.tile([16, N], F32, tag="tk")
                        nc.vector.tensor_mul(tk, s1, s1)
                        nssy = statd.tile([16, N], F32, tag="nssy")
                        nc.vector.scalar_tensor_tensor(
                            out=nssy, in0=tk, scalar=invn_ap, in1=s2,
                            op0=OP.mult, op1=OP.subtract)
                        lssy = statd.tile([16, N], F32, tag="lssy")
                        nc.scalar.activation(out=lssy, in_=nssy, func=AF.Ln,
                                             bias=0.0, scale=-1.0)
                        nc.scalar.activation(out=beta16, in_=lssy,
                                             func=AF.Exp, bias=0.0,
                                             scale=-0.5)
                    else:
                        # mx = s1x/n; a = exp(-.5*ln(ssx))
                        nc.vector.tensor_scalar(out=mx, in0=s1,
                                                scalar1=invn_ap,
                                                scalar2=None, op0=OP.mult)
                        tq = statd.tile([16, N], F32, tag="tq")
                        nc.vector.tensor_mul(tq, s1, s1)
                        nssx = statd.tile([16, N], F32, tag="nssx")
                        nc.vector.scalar_tensor_tensor(
                            out=nssx, in0=tq, scalar=invn_ap, in1=s2,
                            op0=OP.mult, op1=OP.subtract)
                        lssx = statd.tile([16, N], F32, tag="lssx")
                        nc.scalar.activation(out=lssx, in_=nssx, func=AF.Ln,
                                             bias=0.0, scale=-1.0)
                        a16 = statd.tile([16, N], F32, tag="a16")
                        nc.scalar.activation(out=a16, in_=lssx,
                                             func=AF.Exp, bias=0.0,
                                             scale=-0.5)

            # transpose the scale table to [128, 8*16]: col ic*16 + r
            with tc.tile_pool(name="tps", bufs=1, space="PSUM") as tps:
                pst = tps.tile([128, 128], F32, tag="pst_a")
                for ic in range(8):
                    nc.tensor.transpose(pst[:, ic * 16:(ic + 1) * 16],
                                        in_=a16[:, ic * 128:(ic + 1) * 128],
                                        identity=id_s[0:16, 0:16])
                nc.scalar.copy(aT, pst)

        # m1 augmentation: overwrite q_projT row rb+63 (unused d=63) with
        # mx1. ACT-ring DMAs: on the sync ring they would
        # head-of-line-block the yraw copies queued behind them.
        for ch in range(4):
            nc.scalar.dma_start(out=qproj[ch][63:128:64, :],
                                in_=mx[4 * ch:4 * ch + 3:2, :])

        # ---- Stage E: per-head Grams + elementwise (software-pipelined)
        # Flat iteration t = 8*h + ic; quad qd = t//4.
        #   step t+0: PE Gram matmuls -> psg1/psg2
        #   step t+1: evac1 ACT Square -> sgA quarter; evac2 ACT Square or
        #             DVE tensor_scalar (r2=a2*G2) + DVE/Pool f16 square
        #   quad done: c1 = sgA - C, c2 = sgB - C (DVE TS, 4x),
        #              u = c1*c2 (DVE TT, 2x), ACT Ln, DVE -0.5 (4x), store
        T = H * 8
        with tc.tile_pool(name="head", bufs=2) as headp, \
             tc.tile_pool(name="sga", bufs=2) as sgap, \
             tc.tile_pool(name="sgb", bufs=3) as sgbp, \
             tc.tile_pool(name="c1p", bufs=1) as c1p, \
             tc.tile_pool(name="uqp", bufs=2) as uqp, \
             tc.tile_pool(name="ltp", bufs=2) as ltp, \
             tc.tile_pool(name="gps", bufs=3, space="PSUM") as gps, \
             tc.tile_pool(name="bbp", bufs=1, space="PSUM") as bbp:

            SGW = 4 * N  # quad-wide working tiles

            heads = {}

            def prep_head(h):
                prep_y(h)
                prep_x(h)

            def prep_y(h):
                ch, rb = h // 2, (h % 2) * 64
                yo1, yo2 = rb, 64 - rb
                r1, r2 = 2 * h, 2 * h + 1
                # Y raw: m1 block rows yo1..yo1+63 (k d=1..63 + aug),
                #        m2 block rows yo2..yo2+62 (k d=2..63 + aug)
                yraw = headp.tile([128, N], F32R, name="yraw", tag="yraw")
                nc.sync.dma_start(out=yraw[yo1:yo1 + 63, :],
                                  in_=kproj[ch][rb + 1:rb + 64, :])
                nc.sync.dma_start(out=yraw[yo1 + 63:yo1 + 64, :],
                                  in_=ns1y[r1:r1 + 1, :])
                hole = yo2 + 63  # the single uncovered row
                nc.sync.dma_start(out=yraw[hole:hole + 1, :],
                                  in_=ns1y[r1:r1 + 1, :])
                nc.sync.dma_start(out=yraw[yo2:yo2 + 62, :],
                                  in_=kproj[ch][rb + 2:rb + 64, :])
                nc.sync.dma_start(out=yraw[yo2 + 62:yo2 + 63, :],
                                  in_=ns1y[r2:r2 + 1, :])

                # beta broadcast via PE outer product: bb[p, j] =
                # sum_r bbm[r, 128h+p] * beta16[r, j]; bbm has ones at
                # (2h, m1-rows) and (2h+1, m2-rows). Replaces two SWDGE
                # DRAM broadcasts (4 MB of DMA traffic per kernel).
                bb_ps = bbp.tile([128, N], F32, name="bb_ps", tag="bb")
                for jh in range(2):
                    jsl = slice(jh * 512, (jh + 1) * 512)
                    nc.tensor.matmul(bb_ps[:, jsl],
                                     lhsT=bbm_r[:, h * 128:(h + 1) * 128],
                                     rhs=beta16[:, jsl],
                                     start=True, stop=True)
                yt = headp.tile([128, N], F32R, name="yt", tag="yt")
                nc.vector.tensor_mul(yt, yraw, bb_ps)

                heads[h] = [yt, None]

            def prep_x(h):
                ch, rb = h // 2, (h % 2) * 64
                yo2 = 64 - rb
                r2 = 2 * h + 1
                # X2: m2 lhsT block at rows yo2..yo2+62 (q d=0..61 + mx2)
                x2 = headp.tile([128, N], F32R, name="x2", tag="x2")
                nc.sync.dma_start(out=x2[yo2:yo2 + 62, :],
                                  in_=qproj[ch][rb:rb + 62, :])
                nc.sync.dma_start(out=x2[yo2 + 62:yo2 + 63, :],
                                  in_=mx[r2:r2 + 1, :])
                heads[h][1] = x2

            st = {}
            quads = {}

            def emit_pe(t):
                h, ic = divmod(t, 8)
                ch, rb = h // 2, (h % 2) * 64
                yo1, yo2 = rb, 64 - rb
                yt, x2 = heads[h]
                isl = slice(ic * 128, (ic + 1) * 128)
                psg1 = gps.tile([128, N], F32, name="psg1", tag="psg")
                psg2 = gps.tile([128, N], F32, name="psg2", tag="psg")
                for jh in range(2):
                    jsl = slice(jh * 512, (jh + 1) * 512)
                    nc.tensor.matmul(psg1[:, jsl],
                                     lhsT=qproj[ch][rb:rb + 64, isl],
                                     rhs=yt[yo1:yo1 + 64, jsl],
                                     start=True, stop=True)
                    nc.tensor.matmul(psg2[:, jsl],
                                     lhsT=x2[yo2:yo2 + 63, isl],
                                     rhs=yt[yo2:yo2 + 63, jsl],
                                     start=True, stop=True)
                st[t] = (psg1, psg2)

            def emit_evac(t):
                h, ic = divmod(t, 8)
                r1, r2 = 2 * h, 2 * h + 1
                qd, qu = divmod(t, 4)
                psg1, psg2 = st.pop(t)
                if qu == 0:
                    quads[qd] = {
                        "sgA": sgap.tile([128, SGW], F16, name="sgA",
                                         tag="sgA"),
                        "sgB": sgbp.tile([128, SGW], F16, name="sgB",
                                         tag="sgB"),
                        "rB": sgbp.tile([128, SGW], F16, name="rB",
                                        tag="rB"),
                    }
                qw = quads[qd]
                sl = slice(qu * N, (qu + 1) * N)
                a1 = aT[:, ic * 16 + r1:ic * 16 + r1 + 1]
                # v1 = (a1*G1)^2 on ACT
                nc.scalar.activation(out=qw["sgA"][:, sl], in_=psg1,
                                     func=AF.Square, bias=0.0, scale=a1)
                a2 = aT[:, ic * 16 + r2:ic * 16 + r2 + 1]
                if (t % 16) in _ACT_EVAC2_SLOTS:
                    # balance: occasionally v2 = (a2*G2)^2 on ACT
                    nc.scalar.activation(out=qw["sgB"][:, sl], in_=psg2,
                                         func=AF.Square, bias=0.0, scale=a2)
                else:
                    # DVE path: r2 = a2*G2 (PSUM->f16), then square in f16
                    # (only one PSUM operand allowed per instruction)
                    rB = qw["rB"]
                    nc.vector.tensor_scalar(out=rB[:, sl], in0=psg2,
                                            scalar1=a2, scalar2=None,
                                            op0=OP.mult)
                    if t >= 8 and (t % 16) in _POOL_SQ_SLOTS:
                        nc.gpsimd.tensor_mul(qw["sgB"][:, sl], rB[:, sl],
                                             rB[:, sl])
                    else:
                        nc.vector.tensor_mul(qw["sgB"][:, sl], rB[:, sl],
                                             rB[:, sl])

            def emit_combine(qd):
                # c1 = v1 (-) C; c2 = v2 (-) C; u = c1*c2. Both factors use
                # the same subtract convention, so u = +(v1-C)(v2-C) > 0
                # regardless of the ALU's operand order. TS runs 4x and TT
                # 2x; the fused STT alternative has no DVE perf modes.
                qw = quads[qd]
                c1 = c1p.tile([128, SGW], F16, name="c1", tag="c1")
                c2 = c1p.tile([128, SGW], F16, name="c2", tag="c2")
                uq = uqp.tile([128, SGW], F16, name="uq", tag="uq")
                nc.vector.tensor_scalar(out=c1, in0=qw["sgA"], scalar1=C,
                                        scalar2=None, op0=OP.subtract)
                nc.vector.tensor_scalar(out=c2, in0=qw["sgB"], scalar1=C,
                                        scalar2=None, op0=OP.subtract)
                nc.vector.tensor_mul(uq, c1, c2)
                qw["uq"] = uq

            def emit_ln(qd):
                qw = quads[qd]
                lt = ltp.tile([128, 4 * N], F16, name="lt", tag="lt")
                nc.scalar.activation(out=lt, in_=qw["uq"], func=AF.Ln,
                                     bias=0.0, scale=1.0)
                qw["lt"] = lt

            def emit_scale(qd):
                # write the scaled output back into the (now dead) uq tile
                qw = quads[qd]
                eng = (nc.gpsimd if (qd % 4) in _POOL_SCALE_SLOTS
                       else nc.vector)
                eng.tensor_scalar(out=qw["uq"], in0=qw["lt"], scalar1=-0.5,
                                  scalar2=None, op0=OP.mult)
                qw["ow"] = qw["uq"]

            def emit_store(qd):
                qw = quads.pop(qd)
                t0 = 4 * qd
                h, ic0 = divmod(t0, 8)
                dst = out[h, ic0 * 128:(ic0 + 4) * 128, :].rearrange(
                    "(t p) j -> p t j", t=4)
                src = qw["ow"][:, :].rearrange("p (t j) -> p t j", t=4)
                nc.sync.dma_start(out=dst, in_=src)

            prep_head(0)
            prep_head(1)
            for step in range(T + 8):
                if step < T:
                    emit_pe(step)
                    h, ic = divmod(step, 8)
                    if ic == 7 and h + 2 < H:
                        prep_head(h + 2)
                if 0 <= step - 1 < T:
                    emit_evac(step - 1)
                s = step - 3
                if 0 <= s < T and s % 4 == 3:
                    emit_combine((s - 3) // 4)
                s = step - 4
                if 0 <= s < T and s % 4 == 3:
                    emit_ln((s - 3) // 4)
                s = step - 5
                if 0 <= s < T and s % 4 == 3:
                    emit_scale((s - 3) // 4)
                s = step - 6
                if 0 <= s < T and s % 4 == 3:
                    emit_store((s - 3) // 4)


class _ActTablePatch:
    """Force the act-table pass to use natural_log_exp_and_others for every
    activation (it contains Identity/Copy/Square/Ln/Exp). The default
    first-match selection flips between natural_log and exp_and_others,
    costing a ~1.3-2.7us table load per switch. Keys/order are preserved
    (index = act_func_set_id), other sets are just emptied. Scoped so
    other modules building through bacc are unaffected."""

    def __enter__(self):
        self._orig = bacc.get_activation_tables
        keep = "natural_log_exp_and_others"
        orig = self._orig

        def _filtered(arch):
            tabs = orig(arch)
            if keep not in tabs:
                return tabs
            return {k: (v if k == keep else set()) for k, v in tabs.items()}

        bacc.get_activation_tables = _filtered

    def __exit__(self, *exc):
        bacc.get_activation_tables = self._orig


def _build_nc(repeat=1):
    nc = bacc.Bacc("TRN2", target_bir_lowering=False)

    qT = nc.dram_tensor("qT", [DF, N], F32, kind="ExternalInput")
    kT = nc.dram_tensor("kT", [DF, N], F32, kind="ExternalInput")
    wqT = nc.dram_tensor("wqT", [DF, 512], F32, kind="ExternalInput")
    wkT = nc.dram_tensor("wkT", [DF, 512], F32, kind="ExternalInput")
    bq = nc.dram_tensor("bq", [128, 4], F32, kind="ExternalInput")
    bk = nc.dram_tensor("bk", [128, 4], F32, kind="ExternalInput")
    xmask = nc.dram_tensor("xmask", [128, 64], F32, kind="ExternalInput")
    ymask = nc.dram_tensor("ymask", [128, 64], F32, kind="ExternalInput")
    invn = nc.dram_tensor("invn", [16, 1], F32, kind="ExternalInput")
    ident = nc.dram_tensor("ident", [128, 128], F32, kind="ExternalInput")
    out = nc.dram_tensor("out", [H, N, N], F16, kind="ExternalOutput")
    bbm = nc.dram_tensor("bbm", [16, 8 * 128], F32, kind="ExternalInput")

    t = (qT, kT, wqT, wkT, bq, bk, xmask, ymask, invn, ident, out, bbm)
    with tile.TileContext(nc) as tc:
        for _rep in range(repeat):
            _emit_body(nc, tc, t)
    with _ActTablePatch():
        nc.compile()
    return nc


_NC = None


def _get_nc():
    global _NC
    if _NC is None:
        _NC = _build_nc()
    return _NC


def _host_inputs(queries, keys, Wq_w, Wq_b, Wk_w, Wk_b):
    qT = np.ascontiguousarray(queries.transpose(0, 2, 1), dtype=np.float32)
    kT = np.ascontiguousarray(keys.transpose(0, 2, 1), dtype=np.float32)
    wqT = np.ascontiguousarray(Wq_w.T, dtype=np.float32)
    wkT = np.ascontiguousarray(Wk_w.T, dtype=np.float32)
    bq = np.ascontiguousarray(Wq_b.reshape(4, 128).T, dtype=np.float32)
    bk = np.ascontiguousarray(Wk_b.reshape(4, 128).T, dtype=np.float32)

    xmask = np.zeros((128, 64), dtype=np.float32)
    ymask = np.zeros((128, 64), dtype=np.float32)
    for c in range(4):
        for hp in range(2):
            for m in (1, 2):
                j = 4 * c + 2 * hp + (m - 1)      # output partition row r
                col = 16 * c + j                   # column within this chunk's mask
                rows = np.arange(hp * 64, hp * 64 + 64 - m)
                xmask[rows, col] = 1.0
                yrows = np.arange(hp * 64 + m, hp * 64 + 64)
                ymask[yrows, col] = 1.0

    invn = np.array([[1.0 / (64 - ((r % 2) + 1))] for r in range(16)],
                    dtype=np.float32)
    ident = np.eye(128, dtype=np.float32)

    # beta-broadcast masks: bb[p, j] = sum_r bbm[r, 128h+p]*beta[r, j]
    # must equal beta_{2h} on the m1 rows (yo1..yo1+63) and beta_{2h+1}
    # on the m2 rows (yo2..yo2+64, hole included; value there unused)
    bbm = np.zeros((16, 8 * 128), dtype=np.float32)
    for h in range(8):
        rb = (h % 2) * 64
        yo1, yo2 = rb, 64 - rb
        bbm[2 * h, h * 128 + yo1:h * 128 + yo1 + 64] = 1.0
        bbm[2 * h + 1, h * 128 + yo2:h * 128 + yo2 + 64] = 1.0

    shared = dict(wqT=wqT, wkT=wkT, bq=bq, bk=bk, xmask=xmask, ymask=ymask,
                  invn=invn, ident=ident, bbm=bbm)
    in_maps = []
    for b in range(B):
        m = dict(shared)
        m["qT"] = np.ascontiguousarray(qT[b])
        m["kT"] = np.ascontiguousarray(kT[b])
        in_maps.append(m)
    return in_maps


def kernel(queries, keys, Wq_w, Wq_b, Wk_w, Wk_b):
    nc = _get_nc()
    in_maps = _host_inputs(np.asarray(queries), np.asarray(keys),
                           np.asarray(Wq_w), np.asarray(Wq_b),
                           np.asarray(Wk_w), np.asarray(Wk_b))
    res = run_bass_kernel_spmd(nc, in_maps, core_ids=list(range(B)))
    out = np.stack([res.results[b]["out"].astype(np.float32) for b in range(B)],
                   axis=0)
    return out


# revision 41
# speedup vs baseline: 15.2337x; 15.2337x over previous
"""EntropyGraph Trainium2 kernel (v2).

Computes, per batch b (one NeuronCore per batch):
  qt = heads(queries @ Wq_w.T + Wq_b), kt = heads(keys @ Wk_w.T + Wk_b)
  out[b,h,i,j] = -0.5 * sum_m log(1 - corr_m(i,j)^2 + eps)
where corr_m is the lag-m cross-correlation between query series i and key
series j within each head.

Structure vs v1:
  - corr = alpha_i * G[i,j]; G = PE Gram of (raw q rows + mean-aug row)
    against (beta-scaled k rows + -s1y-aug rows). One-sided centering makes
    the mean correction exact.
  - Per iteration t = 8h+ic the two Gram PSUM tiles are evacuated as
      v1 = (a1*G1)^2  (ACT Square, scale=a1 per partition)
      v2: mostly DVE tensor_scalar (r2 = a2*G2, PSUM allows one operand)
          followed by an f16 square on DVE/Pool; ~6/16 slots ride ACT
          Square instead to balance engine load
    then u = (v1-C)(v2-C) as quad-batched [128,4096] f16 DVE ops
    (tensor_scalar 4x + tensor_tensor 2x; scalar_tensor_tensor has no DVE
    perf modes so it is avoided), one ACT Ln per quad, and a -0.5 DVE
    tensor_scalar (4x) written back over the dead u tile.
  - beta broadcast comes from a PE outer product (mask @ beta16) into
    PSUM instead of SWDGE DRAM broadcasts: -4 MB DMA traffic and no
    betad bounce on the startup critical path.
  - All ACT functions (Identity/Copy/Square/Ln/Exp) live in one table set
    (natural_log_exp_and_others, forced via _ActTablePatch); rsqrt is
    computed as exp(-0.5*ln(x)) so no table switches occur.
  - Inputs are cast to f32r so every matmul runs at 1 cycle/row.
  - Prologue runs per side (k fully first: its stats feed beta -> bb ->
    yt which gate head 0) with squared-projection tiles split ACT/DVE.
"""

import sys

import numpy as np

sys.path.insert(0, "/opt/trn_rl_repo")

import concourse.bacc as bacc
import concourse.tile as tile
from concourse import mybir
from concourse.bass_utils import run_bass_kernel_spmd

F32 = mybir.dt.float32
F32R = mybir.dt.float32r
F16 = mybir.dt.float16
OP = mybir.AluOpType
AF = mybir.ActivationFunctionType

B, N, DF = 8, 1024, 128
H, DK = 8, 64
EPS = 1e-6
C = 1.0 + EPS
NCHUNK = 4
# evac2 rides ACT (instead of DVE) when t % 16 in this set: engine balance
_ACT_EVAC2_SLOTS = (1, 3, 6, 9, 12, 14)
# on the DVE evac2 path, the f16 squaring op goes to Pool when t % 16 in
# this set (Pool Multiply eff 0.42 but otherwise idle in steady state)
_POOL_SQ_SLOTS = (0, 2, 4, 5, 7, 8, 10, 11, 13, 15)
# -0.5 scale quads routed to Pool when qd % 4 in this set
_POOL_SCALE_SLOTS = ()


def _emit_body(nc, tc, t):
    qT, kT, wqT, wkT, bq, bk, xmask, ymask, invn, ident, out, bbm = t
    with tc.tile_pool(name="const", bufs=1) as const, \
         tc.tile_pool(name="proj", bufs=1) as projp, \
         tc.tile_pool(name="stats", bufs=1) as statp:

        # statp: tiles that stage E reads; everything else transient.
        ns1y = statp.tile([16, N], F32R)
        mx = statp.tile([16, N], F32R)
        aT = statp.tile([128, 128], F32)
        beta16 = statp.tile([16, N], F32R)

        invn_s = const.tile([16, 1], F32)
        id_s = const.tile([128, 128], F32)
        bbm_r = const.tile([16, 8 * 128], F32R)

        with tc.tile_pool(name="inp", bufs=1) as inp, \
             tc.tile_pool(name="statd", bufs=1) as statd:
            # ---- Stage A: load inputs ---------------------------------
            qT_s = inp.tile([DF, N], F32)
            kT_s = inp.tile([DF, N], F32)
            wqT_s = inp.tile([DF, 512], F32)
            wkT_s = inp.tile([DF, 512], F32)
            bq_s = inp.tile([128, 4], F32)
            bk_s = inp.tile([128, 4], F32)
            xm_s = inp.tile([128, 64], F32)
            ym_s = inp.tile([128, 64], F32)
            bbm_s = inp.tile([16, 8 * 128], F32)
            for dst, src in ((kT_s, kT), (wkT_s, wkT), (qT_s, qT),
                             (wqT_s, wqT), (bq_s, bq), (bk_s, bk),
                             (xm_s, xmask), (ym_s, ymask), (invn_s, invn),
                             (id_s, ident), (bbm_s, bbm)):
                nc.sync.dma_start(out=dst, in_=src[:, :])

            # f32r rounding casts (the verifier rejects raw-DMA data as
            # f32r matmul input). k-side first: beta gates stage E head 0.
            kT_r = inp.tile([DF, N], F32R)
            wkT_r = inp.tile([DF, 512], F32R)
            qT_r = inp.tile([DF, N], F32R)
            wqT_r = inp.tile([DF, 512], F32R)
            xm_r = inp.tile([128, 64], F32R)
            ym_r = inp.tile([128, 64], F32R)
            nc.vector.tensor_copy(kT_r, kT_s)
            nc.scalar.copy(wkT_r, wkT_s)
            nc.vector.tensor_copy(qT_r, qT_s)
            nc.scalar.copy(wqT_r, wqT_s)
            nc.scalar.copy(ym_r, ym_s)
            nc.scalar.copy(xm_r, xm_s)
            nc.scalar.copy(bbm_r, bbm_s)

            # ---- Stages B-D, one side at a time ----------------------
            # Each side runs proj -> sq -> moment matmuls -> stats evac ->
            # stage-D math end-to-end, k-side first: the k chain feeds
            # beta16 -> bb -> yt (head 0 Gram rhs) while the q side is
            # still projecting, and every engine queue sees the k-chain
            # ops first. sq ops stay off Pool so bb broadcasts are not
            # stuck behind 2.1us Pool multiplies.
            qproj = []
            kproj = []
            stats_sb = {}
            with tc.tile_pool(name="sqp", bufs=1) as sqp, \
                 tc.tile_pool(name="sps", bufs=1, space="PSUM") as sps:
                for (src_r, w_r, b_s, mask, dst_list, pname) in (
                        (kT_r, wkT_r, bk_s, ym_r, kproj, "k"),
                        (qT_r, wqT_r, bq_s, xm_r, qproj, "q")):
                    sq_side = []
                    with tc.tile_pool(name=f"pps{pname}", bufs=2,
                                      space="PSUM") as pps:
                        for c in range(NCHUNK):
                            psb = pps.tile([128, N], F32, tag="pps")
                            for jh in range(2):
                                nc.tensor.matmul(
                                    psb[:, jh * 512:(jh + 1) * 512],
                                    lhsT=w_r[:, c * 128:(c + 1) * 128],
                                    rhs=src_r[:, jh * 512:(jh + 1) * 512],
                                    start=True, stop=True)
                            pt = projp.tile([128, N], F32R,
                                            tag=f"proj_{pname}_{c}")
                            if c % 2 == 1:
                                nc.vector.tensor_scalar(
                                    out=pt, in0=psb, scalar1=1.0,
                                    scalar2=b_s[:, c:c + 1],
                                    op0=OP.mult, op1=OP.add)
                            else:
                                nc.scalar.activation(
                                    out=pt, in_=psb, func=AF.Identity,
                                    bias=b_s[:, c:c + 1], scale=1.0)
                            dst_list.append(pt)
                            sq = sqp.tile([128, N], F32R,
                                          tag=f"sq{pname}{c}")
                            if c % 2 == 0:
                                nc.vector.tensor_mul(sq, pt, pt)
                            else:
                                nc.scalar.activation(
                                    out=sq, in_=pt, func=AF.Square,
                                    bias=0.0, scale=1.0)
                            sq_side.append(sq)

                    # moment matmuls after all chunks: keeps PE's matmul
                    # bursts dense (p-state ramps to full speed only after
                    # ~3us of continuous work)
                    ps1 = sps.tile([16, N], F32, tag="ps1")
                    ps2 = sps.tile([16, N], F32, tag="ps2")
                    for c in range(NCHUNK):
                        for jh in range(2):
                            sl = slice(jh * 512, (jh + 1) * 512)
                            nc.tensor.matmul(
                                ps1[:, sl],
                                lhsT=mask[:, 16 * c:16 * c + 16],
                                rhs=dst_list[c][:, sl],
                                start=(c == 0), stop=(c == NCHUNK - 1))
                            nc.tensor.matmul(
                                ps2[:, sl],
                                lhsT=mask[:, 16 * c:16 * c + 16],
                                rhs=sq_side[c][:, sl],
                                start=(c == 0), stop=(c == NCHUNK - 1))

                    s1 = statd.tile([16, N], F32, tag=f"s1{pname}")
                    s2 = statd.tile([16, N], F32, tag=f"s2{pname}")
                    nc.scalar.copy(s1, ps1)
                    nc.vector.tensor_copy(s2, ps2)
                    stats_sb[pname] = (s1, s2)

                    invn_ap = invn_s[:, 0:1]
                    if pname == "k":
                        # nssy = s1y^2/n - s2y = -ssy; beta = exp(-.5 ln ssy)
                        nc.vector.tensor_scalar(out=ns1y, in0=s1,
                                                scalar1=-1.0, scalar2=None,
                                                op0=OP.mult)
                        tk = statd.tile([16, N], F32, tag="tk")
                        nc.vector.tensor_mul(tk, s1, s1)
                        nssy = statd.tile([16, N], F32, tag="nssy")
                        nc.vector.scalar_tensor_tensor(
                            out=nssy, in0=tk, scalar=invn_ap, in1=s2,
                            op0=OP.mult, op1=OP.subtract)
                        lssy = statd.tile([16, N], F32, tag="lssy")
                        nc.scalar.activation(out=lssy, in_=nssy, func=AF.Ln,
                                             bias=0.0, scale=-1.0)
                        nc.scalar.activation(out=beta16, in_=lssy,
                                             func=AF.Exp, bias=0.0,
                                             scale=-0.5)
                    else:
                        # mx = s1x/n; a = exp(-.5*ln(ssx))
                        nc.vector.tensor_scalar(out=mx, in0=s1,
                                                scalar1=invn_ap,
                                                scalar2=None, op0=OP.mult)
                        tq = statd.tile([16, N], F32, tag="tq")
                        nc.vector.tensor_mul(tq, s1, s1)
                        nssx = statd.tile([16, N], F32, tag="nssx")
                        nc.vector.scalar_tensor_tensor(
                            out=nssx, in0=tq, scalar=invn_ap, in1=s2,
                            op0=OP.mult, op1=OP.subtract)
                        lssx = statd.tile([16, N], F32, tag="lssx")
                        nc.scalar.activation(out=lssx, in_=nssx, func=AF.Ln,
                                             bias=0.0, scale=-1.0)
                        a16 = statd.tile([16, N], F32, tag="a16")
                        nc.scalar.activation(out=a16, in_=lssx,
                                             func=AF.Exp, bias=0.0,
                                             scale=-0.5)

            # transpose the scale table to [128, 8*16]: col ic*16 + r
            with tc.tile_pool(name="tps", bufs=1, space="PSUM") as tps:
                pst = tps.tile([128, 128], F32, tag="pst_a")
                for ic in range(8):
                    nc.tensor.transpose(pst[:, ic * 16:(ic + 1) * 16],
                                        in_=a16[:, ic * 128:(ic + 1) * 128],
                                        identity=id_s[0:16, 0:16])
                nc.scalar.copy(aT, pst)

        # m1 augmentation: overwrite q_projT row rb+63 (unused d=63) with
        # mx1. ACT-ring DMAs: on the sync ring they would
        # head-of-line-block the yraw copies queued behind them.
        for ch in range(4):
            nc.scalar.dma_start(out=qproj[ch][63:128:64, :],
                                in_=mx[4 * ch:4 * ch + 3:2, :])

        # ---- Stage E: per-head Grams + elementwise (software-pipelined)
        # Flat iteration t = 8*h + ic; quad qd = t//4.
        #   step t+0: PE Gram matmuls -> psg1/psg2
        #   step t+1: evac1 ACT Square -> sgA quarter; evac2 ACT Square or
        #             DVE tensor_scalar (r2=a2*G2) + DVE/Pool f16 square
        #   quad done: c1 = sgA - C, c2 = sgB - C (DVE TS, 4x),
        #              u = c1*c2 (DVE TT, 2x), ACT Ln, DVE -0.5 (4x), store
        T = H * 8
        with tc.tile_pool(name="head", bufs=2) as headp, \
             tc.tile_pool(name="sga", bufs=2) as sgap, \
             tc.tile_pool(name="sgb", bufs=3) as sgbp, \
             tc.tile_pool(name="c1p", bufs=1) as c1p, \
             tc.tile_pool(name="uqp", bufs=2) as uqp, \
             tc.tile_pool(name="ltp", bufs=2) as ltp, \
             tc.tile_pool(name="gps", bufs=3, space="PSUM") as gps, \
             tc.tile_pool(name="bbp", bufs=1, space="PSUM") as bbp:

            SGW = 4 * N  # quad-wide working tiles

            heads = {}

            def prep_head(h):
                prep_y(h)
                prep_x(h)

            def prep_y(h):
                ch, rb = h // 2, (h % 2) * 64
                yo1, yo2 = rb, 64 - rb
                r1, r2 = 2 * h, 2 * h + 1
                # Y raw: m1 block rows yo1..yo1+63 (k d=1..63 + aug),
                #        m2 block rows yo2..yo2+62 (k d=2..63 + aug)
                yraw = headp.tile([128, N], F32R, name="yraw", tag="yraw")
                nc.sync.dma_start(out=yraw[yo1:yo1 + 63, :],
                                  in_=kproj[ch][rb + 1:rb + 64, :])
                nc.sync.dma_start(out=yraw[yo1 + 63:yo1 + 64, :],
                                  in_=ns1y[r1:r1 + 1, :])
                hole = yo2 + 63  # the single uncovered row
                nc.sync.dma_start(out=yraw[hole:hole + 1, :],
                                  in_=ns1y[r1:r1 + 1, :])
                nc.sync.dma_start(out=yraw[yo2:yo2 + 62, :],
                                  in_=kproj[ch][rb + 2:rb + 64, :])
                nc.sync.dma_start(out=yraw[yo2 + 62:yo2 + 63, :],
                                  in_=ns1y[r2:r2 + 1, :])

                # beta broadcast via PE outer product: bb[p, j] =
                # sum_r bbm[r, 128h+p] * beta16[r, j]; bbm has ones at
                # (2h, m1-rows) and (2h+1, m2-rows). Replaces two SWDGE
                # DRAM broadcasts (4 MB of DMA traffic per kernel).
                bb_ps = bbp.tile([128, N], F32, name="bb_ps", tag="bb")
                for jh in range(2):
                    jsl = slice(jh * 512, (jh + 1) * 512)
                    nc.tensor.matmul(bb_ps[:, jsl],
                                     lhsT=bbm_r[:, h * 128:(h + 1) * 128],
                                     rhs=beta16[:, jsl],
                                     start=True, stop=True)
                yt = headp.tile([128, N], F32R, name="yt", tag="yt")
                nc.vector.tensor_mul(yt, yraw, bb_ps)

                heads[h] = [yt, None]

            def prep_x(h):
                ch, rb = h // 2, (h % 2) * 64
                yo2 = 64 - rb
                r2 = 2 * h + 1
                # X2: m2 lhsT block at rows yo2..yo2+62 (q d=0..61 + mx2)
                x2 = headp.tile([128, N], F32R, name="x2", tag="x2")
                nc.sync.dma_start(out=x2[yo2:yo2 + 62, :],
                                  in_=qproj[ch][rb:rb + 62, :])
                nc.sync.dma_start(out=x2[yo2 + 62:yo2 + 63, :],
                                  in_=mx[r2:r2 + 1, :])
                heads[h][1] = x2

            st = {}
            quads = {}

            def emit_pe(t):
                h, ic = divmod(t, 8)
                ch, rb = h // 2, (h % 2) * 64
                yo1, yo2 = rb, 64 - rb
                yt, x2 = heads[h]
                isl = slice(ic * 128, (ic + 1) * 128)
                psg1 = gps.tile([128, N], F32, name="psg1", tag="psg")
                psg2 = gps.tile([128, N], F32, name="psg2", tag="psg")
                for jh in range(2):
                    jsl = slice(jh * 512, (jh + 1) * 512)
                    nc.tensor.matmul(psg1[:, jsl],
                                     lhsT=qproj[ch][rb:rb + 64, isl],
                                     rhs=yt[yo1:yo1 + 64, jsl],
                                     start=True, stop=True)
                    nc.tensor.matmul(psg2[:, jsl],
                                     lhsT=x2[yo2:yo2 + 63, isl],
                                     rhs=yt[yo2:yo2 + 63, jsl],
                                     start=True, stop=True)
                st[t] = (psg1, psg2)

            def emit_evac(t):
                h, ic = divmod(t, 8)
                r1, r2 = 2 * h, 2 * h + 1
                qd, qu = divmod(t, 4)
                psg1, psg2 = st.pop(t)
                if qu == 0:
                    quads[qd] = {
                        "sgA": sgap.tile([128, SGW], F16, name="sgA",
                                         tag="sgA"),
                        "sgB": sgbp.tile([128, SGW], F16, name="sgB",
                                         tag="sgB"),
                        "rB": sgbp.tile([128, SGW], F16, name="rB",
                                        tag="rB"),
                    }
                qw = quads[qd]
                sl = slice(qu * N, (qu + 1) * N)
                a1 = aT[:, ic * 16 + r1:ic * 16 + r1 + 1]
                # v1 = (a1*G1)^2 on ACT
                nc.scalar.activation(out=qw["sgA"][:, sl], in_=psg1,
                                     func=AF.Square, bias=0.0, scale=a1)
                a2 = aT[:, ic * 16 + r2:ic * 16 + r2 + 1]
                if (t % 16) in _ACT_EVAC2_SLOTS:
                    # balance: occasionally v2 = (a2*G2)^2 on ACT
                    nc.scalar.activation(out=qw["sgB"][:, sl], in_=psg2,
                                         func=AF.Square, bias=0.0, scale=a2)
                else:
                    # DVE path: r2 = a2*G2 (PSUM->f16), then square in f16
                    # (only one PSUM operand allowed per instruction)
                    rB = qw["rB"]
                    nc.vector.tensor_scalar(out=rB[:, sl], in0=psg2,
                                            scalar1=a2, scalar2=None,
                                            op0=OP.mult)
                    if t >= 8 and (t % 16) in _POOL_SQ_SLOTS:
                        nc.gpsimd.tensor_mul(qw["sgB"][:, sl], rB[:, sl],
                                             rB[:, sl])
                    else:
                        nc.vector.tensor_mul(qw["sgB"][:, sl], rB[:, sl],
                                             rB[:, sl])

            def emit_combine(qd):
                # c1 = v1 (-) C; c2 = v2 (-) C; u = c1*c2. Both factors use
                # the same subtract convention, so u = +(v1-C)(v2-C) > 0
                # regardless of the ALU's operand order. TS runs 4x and TT
                # 2x; the fused STT alternative has no DVE perf modes.
                qw = quads[qd]
                c1 = c1p.tile([128, SGW], F16, name="c1", tag="c1")
                c2 = c1p.tile([128, SGW], F16, name="c2", tag="c2")
                uq = uqp.tile([128, SGW], F16, name="uq", tag="uq")
                nc.vector.tensor_scalar(out=c1, in0=qw["sgA"], scalar1=C,
                                        scalar2=None, op0=OP.subtract)
                nc.vector.tensor_scalar(out=c2, in0=qw["sgB"], scalar1=C,
                                        scalar2=None, op0=OP.subtract)
                nc.vector.tensor_mul(uq, c1, c2)
                qw["uq"] = uq

            def emit_ln(qd):
                qw = quads[qd]
                lt = ltp.tile([128, 4 * N], F16, name="lt", tag="lt")
                if qd == 15:
                    # last quad: halves, so scale/store drain sooner
                    nc.scalar.activation(out=lt[:, :2 * N],
                                         in_=qw["uq"][:, :2 * N],
                                         func=AF.Ln, bias=0.0, scale=1.0)
                    nc.scalar.activation(out=lt[:, 2 * N:],
                                         in_=qw["uq"][:, 2 * N:],
                                         func=AF.Ln, bias=0.0, scale=1.0)
                else:
                    nc.scalar.activation(out=lt, in_=qw["uq"], func=AF.Ln,
                                         bias=0.0, scale=1.0)
                qw["lt"] = lt

            def emit_scale(qd):
                # write the scaled output back into the (now dead) uq tile
                qw = quads[qd]
                eng = (nc.gpsimd if (qd % 4) in _POOL_SCALE_SLOTS
                       else nc.vector)
                if qd == 15:
                    eng.tensor_scalar(out=qw["uq"][:, :2 * N],
                                      in0=qw["lt"][:, :2 * N], scalar1=-0.5,
                                      scalar2=None, op0=OP.mult)
                    eng.tensor_scalar(out=qw["uq"][:, 2 * N:],
                                      in0=qw["lt"][:, 2 * N:], scalar1=-0.5,
                                      scalar2=None, op0=OP.mult)
                else:
                    eng.tensor_scalar(out=qw["uq"], in0=qw["lt"],
                                      scalar1=-0.5, scalar2=None,
                                      op0=OP.mult)
                qw["ow"] = qw["uq"]

            def emit_store(qd):
                qw = quads.pop(qd)
                t0 = 4 * qd
                h, ic0 = divmod(t0, 8)
                if qd == 15:
                    for half in range(2):
                        dst = out[h, (ic0 + 2 * half) * 128:
                                  (ic0 + 2 * half + 2) * 128, :].rearrange(
                            "(t p) j -> p t j", t=2)
                        srcw = qw["ow"][:, 2 * half * N:
                                        (2 * half + 2) * N].rearrange(
                            "p (t j) -> p t j", t=2)
                        nc.sync.dma_start(out=dst, in_=srcw)
                    return
                dst = out[h, ic0 * 128:(ic0 + 4) * 128, :].rearrange(
                    "(t p) j -> p t j", t=4)
                src = qw["ow"][:, :].rearrange("p (t j) -> p t j", t=4)
                nc.sync.dma_start(out=dst, in_=src)

            prep_head(0)
            prep_head(1)
            for step in range(T + 8):
                if step < T:
                    emit_pe(step)
                    h, ic = divmod(step, 8)
                    if ic == 7 and h + 2 < H:
                        prep_head(h + 2)
                if 0 <= step - 1 < T:
                    emit_evac(step - 1)
                s = step - 3
                if 0 <= s < T and s % 4 == 3:
                    emit_combine((s - 3) // 4)
                s = step - 4
                if 0 <= s < T and s % 4 == 3:
                    emit_ln((s - 3) // 4)
                s = step - 5
                if 0 <= s < T and s % 4 == 3:
                    emit_scale((s - 3) // 4)
                s = step - 6
                if 0 <= s < T and s % 4 == 3:
                    emit_store((s - 3) // 4)


class _ActTablePatch:
    """Force the act-table pass to use natural_log_exp_and_others for every
    activation (it contains Identity/Copy/Square/Ln/Exp). The default
    first-match selection flips between natural_log and exp_and_others,
    costing a ~1.3-2.7us table load per switch. Keys/order are preserved
    (index = act_func_set_id), other sets are just emptied. Scoped so
    other modules building through bacc are unaffected."""

    def __enter__(self):
        self._orig = bacc.get_activation_tables
        keep = "natural_log_exp_and_others"
        orig = self._orig

        def _filtered(arch):
            tabs = orig(arch)
            if keep not in tabs:
                return tabs
            return {k: (v if k == keep else set()) for k, v in tabs.items()}

        bacc.get_activation_tables = _filtered

    def __exit__(self, *exc):
        bacc.get_activation_tables = self._orig


def _build_nc(repeat=1):
    nc = bacc.Bacc("TRN2", target_bir_lowering=False)

    qT = nc.dram_tensor("qT", [DF, N], F32, kind="ExternalInput")
    kT = nc.dram_tensor("kT", [DF, N], F32, kind="ExternalInput")
    wqT = nc.dram_tensor("wqT", [DF, 512], F32, kind="ExternalInput")
    wkT = nc.dram_tensor("wkT", [DF, 512], F32, kind="ExternalInput")
    bq = nc.dram_tensor("bq", [128, 4], F32, kind="ExternalInput")
    bk = nc.dram_tensor("bk", [128, 4], F32, kind="ExternalInput")
    xmask = nc.dram_tensor("xmask", [128, 64], F32, kind="ExternalInput")
    ymask = nc.dram_tensor("ymask", [128, 64], F32, kind="ExternalInput")
    invn = nc.dram_tensor("invn", [16, 1], F32, kind="ExternalInput")
    ident = nc.dram_tensor("ident", [128, 128], F32, kind="ExternalInput")
    out = nc.dram_tensor("out", [H, N, N], F16, kind="ExternalOutput")
    bbm = nc.dram_tensor("bbm", [16, 8 * 128], F32, kind="ExternalInput")

    t = (qT, kT, wqT, wkT, bq, bk, xmask, ymask, invn, ident, out, bbm)
    with tile.TileContext(nc) as tc:
        for _rep in range(repeat):
            _emit_body(nc, tc, t)
    with _ActTablePatch():
        nc.compile()
    return nc


_NC = None


def _get_nc():
    global _NC
    if _NC is None:
        _NC = _build_nc()
    return _NC


def _host_inputs(queries, keys, Wq_w, Wq_b, Wk_w, Wk_b):
    qT = np.ascontiguousarray(queries.transpose(0, 2, 1), dtype=np.float32)
    kT = np.ascontiguousarray(keys.transpose(0, 2, 1), dtype=np.float32)
    wqT = np.ascontiguousarray(Wq_w.T, dtype=np.float32)
    wkT = np.ascontiguousarray(Wk_w.T, dtype=np.float32)
    bq = np.ascontiguousarray(Wq_b.reshape(4, 128).T, dtype=np.float32)
    bk = np.ascontiguousarray(Wk_b.reshape(4, 128).T, dtype=np.float32)

    xmask = np.zeros((128, 64), dtype=np.float32)
    ymask = np.zeros((128, 64), dtype=np.float32)
    for c in range(4):
        for hp in range(2):
            for m in (1, 2):
                j = 4 * c + 2 * hp + (m - 1)      # output partition row r
                col = 16 * c + j                   # column within this chunk's mask
                rows = np.arange(hp * 64, hp * 64 + 64 - m)
                xmask[rows, col] = 1.0
                yrows = np.arange(hp * 64 + m, hp * 64 + 64)
                ymask[yrows, col] = 1.0

    invn = np.array([[1.0 / (64 - ((r % 2) + 1))] for r in range(16)],
                    dtype=np.float32)
    ident = np.eye(128, dtype=np.float32)

    # beta-broadcast masks: bb[p, j] = sum_r bbm[r, 128h+p]*beta[r, j]
    # must equal beta_{2h} on the m1 rows (yo1..yo1+63) and beta_{2h+1}
    # on the m2 rows (yo2..yo2+64, hole included; value there unused)
    bbm = np.zeros((16, 8 * 128), dtype=np.float32)
    for h in range(8):
        rb = (h % 2) * 64
        yo1, yo2 = rb, 64 - rb
        bbm[2 * h, h * 128 + yo1:h * 128 + yo1 + 64] = 1.0
        bbm[2 * h + 1, h * 128 + yo2:h * 128 + yo2 + 64] = 1.0

    shared = dict(wqT=wqT, wkT=wkT, bq=bq, bk=bk, xmask=xmask, ymask=ymask,
                  invn=invn, ident=ident, bbm=bbm)
    in_maps = []
    for b in range(B):
        m = dict(shared)
        m["qT"] = np.ascontiguousarray(qT[b])
        m["kT"] = np.ascontiguousarray(kT[b])
        in_maps.append(m)
    return in_maps


def kernel(queries, keys, Wq_w, Wq_b, Wk_w, Wk_b):
    nc = _get_nc()
    in_maps = _host_inputs(np.asarray(queries), np.asarray(keys),
                           np.asarray(Wq_w), np.asarray(Wq_b),
                           np.asarray(Wk_w), np.asarray(Wk_b))
    res = run_bass_kernel_spmd(nc, in_maps, core_ids=list(range(B)))
    out = np.stack([res.results[b]["out"].astype(np.float32) for b in range(B)],
                   axis=0)
    return out
